# revision 11
# baseline (speedup 1.0000x reference)
"""Tensor-parallel Llama attention for 8 TRN2 NeuronCores.

Sharding: core d handles batch d//4 and q-head group g = d%4 (q heads
4g..4g+3, kv head g — GQA group-aligned so each core needs exactly one
kv head).  Wq/Wk/Wv are row-sharded, Wo column-sharded; the per-batch
partial o_proj outputs of 4 cores are summed on the host.

Device layouts (prepared host-side, bf16):
  hsT  [16,128,S]   hidden_states[b].T, HID on partitions in 16 chunks
  wqT  [16,128,512] Wq_shard.T          wkT/wvT [16,128,128]
  woT  [4,128,2048] Wo_shard.T (4 contraction chunks of the 512 local dims)
  cosT/sinT [128,S] RoPE tables in [head_dim, seq] layout
  mask [4,128,512]  0/1 causal masks for the 4 diagonal-block phases

Compute: q/k kept transposed [d, s] for scores; v transposed back to
[s, d] via PE transpose for PV; scores computed as scoresT [j, i] so
softmax probs feed PV directly without transposition.  Softmax sums via
ones-vector matmul over partitions (no max subtraction: inputs are
N(0,~0.8) scores, exp stays far below f32 overflow).
"""

import sys

sys.path.insert(0, "/opt/trn_rl_repo")

import numpy as np
import ml_dtypes

B, S, HID = 2, 2048, 2048
NH, NKV, HD = 16, 4, 128
THETA = 10000.0
NCORES = 8
HPC = 4            # q heads per core
QDIM = HPC * HD    # 512 local q dims
KT = HID // 128    # 16 contraction chunks
SB = S // 512      # 4 column groups of 512
ST = S // 128      # 16 row tiles of 128

_CACHE = {}


def _patch_tile_drain():
    """This walrus build caps sync waits per CTRL instruction below what the
    stock Tile kernel-tail drain carries; split them into single-wait NOPs."""
    import bass_rust
    import concourse.tile as tile
    from concourse.tile import ScopedClock

    if getattr(tile.TileContext, "_drain_split_patched", False):
        return

    def _split_drain_and_barrier(self, tick_clock, wait_clock):
        ticks = list(tick_clock.global_clock)
        for i, v in enumerate(ticks):
            if v > 0:
                single = [0] * len(ticks)
                single[i] = v
                nop = self.nc.sync.nop(nofuse=True, hint=f"drain_wait_{i}")
                wait_clock.add_sem_waits(
                    nop.ins, ScopedClock({None: bass_rust.VectorClock(single)})
                )
        self.nc.sync.drain()
        self.nc.all_engine_barrier()
        assert self.sems is not None
        popped = self.nc._tile_sem_poison_stack.pop()
        assert popped is self._sem_poison
        self.nc.clear_and_free_semaphores(list(self.sems.allocated().values()))
        self.nc.all_engine_barrier()

    tile.TileContext._drain_and_barrier = _split_drain_and_barrier
    tile.TileContext._drain_split_patched = True


def _legalize_waits(nc, max_waits=1):
    """This walrus build rejects instructions carrying more than ~2 sync
    waits.  Hoist the excess onto single-wait NOPs inserted just before the
    instruction in its block (same engine => same instruction stream, so
    the waits still complete before the op issues)."""
    import concourse.mybir as mybir

    n_split = 0
    for block in nc.m.functions[0].blocks:
        insts = list(block.instructions)
        out = []
        for inst in insts:
            si = getattr(inst, "sync_info", None)
            if si is not None and si.on_wait and len(si.on_wait) > max_waits:
                waits = list(si.on_wait)
                keep = waits[:max_waits]
                for j, w in enumerate(waits[max_waits:]):
                    out.append(
                        mybir.InstNoOp(
                            name=f"{inst.name}_hw{j}",
                            engine=inst.engine,
                            bass_nofuse=True,
                            sync_info=mybir.SyncInfo(on_wait=[w], on_update=[]),
                        )
                    )
                si.on_wait = keep
                n_split += 1
            out.append(inst)
        block.instructions = out
    return n_split


def _build_nc():
    import concourse.bass as bass
    import concourse.mybir as mybir
    import concourse.tile as tile
    from concourse.masks import make_identity

    _patch_tile_drain()

    bf = mybir.dt.bfloat16
    f32 = mybir.dt.float32
    Exp = mybir.ActivationFunctionType.Exp

    nc = bass.Bass()
    hsT = nc.declare_dram_parameter("hsT", [KT, 128, S], bf, isOutput=False)
    wqT = nc.declare_dram_parameter("wqT", [KT, 128, QDIM], bf, isOutput=False)
    wkT = nc.declare_dram_parameter("wkT", [KT, 128, HD], bf, isOutput=False)
    wvT = nc.declare_dram_parameter("wvT", [KT, 128, HD], bf, isOutput=False)
    woT = nc.declare_dram_parameter("woT", [4, 128, HID], bf, isOutput=False)
    cosT = nc.declare_dram_parameter("cosT", [128, S], bf, isOutput=False)
    sinT = nc.declare_dram_parameter("sinT", [128, S], bf, isOutput=False)
    mask = nc.declare_dram_parameter("mask", [4, 128, 512], bf, isOutput=False)
    out = nc.declare_dram_parameter("out", [S, HID], f32, isOutput=True)

    with tile.TileContext(nc) as tc:
        with (
            tc.tile_pool(name="resid", bufs=1) as resid,
            tc.tile_pool(name="probs", bufs=4) as probs_pool,
            tc.tile_pool(name="rc", bufs=2) as rc_pool,
            tc.tile_pool(name="bc", bufs=2) as bc_pool,
            tc.tile_pool(name="ostage", bufs=4) as ostage_pool,
            tc.tile_pool(name="mm_ps", bufs=2, space="PSUM") as mm_ps,
            tc.tile_pool(name="score_ps", bufs=2, space="PSUM") as score_ps,
            tc.tile_pool(name="pv_ps", bufs=2, space="PSUM") as pv_ps,
            tc.tile_pool(name="sum_ps", bufs=2, space="PSUM") as sum_ps,
        ):
            hs_sb = resid.tile([128, KT * S], bf)
            wq_sb = resid.tile([128, KT * QDIM], bf)
            wk_sb = resid.tile([128, KT * HD], bf)
            wv_sb = resid.tile([128, KT * HD], bf)
            wo_sb = resid.tile([128, 4 * HID], bf)
            cos_sb = resid.tile([128, S], bf)
            sin_sb = resid.tile([128, S], bf)
            mask_sb = resid.tile([128, 4 * 512], bf)
            ones_sb = resid.tile([128, 1], bf)
            onerow_sb = resid.tile([1, 128], f32)
            ident = resid.tile([128, 128], bf)
            qT_sb = resid.tile([128, HPC * S], bf)
            kT_sb = resid.tile([128, S], bf)
            vT_sb = resid.tile([128, S], bf)
            vn_sb = resid.tile([128, S], bf)
            at_sb = resid.tile([128, HPC * S], bf)
            rot_sb = resid.tile([128, S], bf)

            # ---- load everything ----
            for kk in range(KT):
                nc.sync.dma_start(hs_sb[:, kk * S:(kk + 1) * S], hsT[kk])
                nc.sync.dma_start(wq_sb[:, kk * QDIM:(kk + 1) * QDIM], wqT[kk])
                nc.sync.dma_start(wk_sb[:, kk * HD:(kk + 1) * HD], wkT[kk])
                nc.sync.dma_start(wv_sb[:, kk * HD:(kk + 1) * HD], wvT[kk])
            for c in range(4):
                nc.sync.dma_start(wo_sb[:, c * HID:(c + 1) * HID], woT[c])
                nc.sync.dma_start(mask_sb[:, c * 512:(c + 1) * 512], mask[c])
            nc.sync.dma_start(cos_sb[:], cosT[:])
            nc.sync.dma_start(sin_sb[:], sinT[:])
            nc.gpsimd.memset(ones_sb[:], 1.0)
            nc.gpsimd.memset(onerow_sb[:], 1.0)
            make_identity(nc, ident[:])

            # ---- q/k/v projections (transposed layouts) ----
            def project(w_sb, wdim, mtiles, dst, dst_stride):
                # dst[:, m*dst_stride + sg*512 ...] = (W.T chunk m) over s
                for m in range(mtiles):
                    for sg in range(SB):
                        ps = mm_ps.tile([128, 512], f32, tag="mm")
                        for kk in range(KT):
                            nc.tensor.matmul(
                                ps[:],
                                w_sb[:, kk * wdim + m * 128: kk * wdim + (m + 1) * 128],
                                hs_sb[:, kk * S + sg * 512: kk * S + sg * 512 + 512],
                                start=(kk == 0),
                                stop=(kk == KT - 1),
                            )
                        nc.vector.tensor_copy(
                            dst[:, m * dst_stride + sg * 512: m * dst_stride + sg * 512 + 512],
                            ps[:],
                        )

            project(wq_sb, QDIM, HPC, qT_sb, S)
            project(wk_sb, HD, 1, kT_sb, S)
            project(wv_sb, HD, 1, vT_sb, S)

            # ---- RoPE (in place, [d, s] layout) ----
            def rope(h):
                nc.vector.tensor_scalar_mul(rot_sb[0:64, :], h[64:128, :], -1.0)
                nc.vector.tensor_copy(rot_sb[64:128, :], h[0:64, :])
                nc.vector.tensor_mul(h, h, cos_sb[:])
                nc.vector.tensor_mul(rot_sb[:], rot_sb[:], sin_sb[:])
                nc.vector.tensor_add(h, h, rot_sb[:])

            for h in range(HPC):
                rope(qT_sb[:, h * S:(h + 1) * S])
            rope(kT_sb[:])

            # ---- v back to natural [s, d] layout via PE transpose ----
            for tj in range(ST):
                tp = mm_ps.tile([128, 128], bf, tag="mm")
                nc.tensor.transpose(tp[:], vT_sb[:, tj * 128:(tj + 1) * 128], ident[:])
                nc.vector.tensor_copy(vn_sb[:, tj * 128:(tj + 1) * 128], tp[:])

            # ---- attention: scoresT -> exp -> mask -> sums + PV ----
            inv_sqrt_d = 1.0 / float(np.sqrt(HD))
            for h in range(HPC):
                qh = qT_sb[:, h * S:(h + 1) * S]
                for gi in range(SB):
                    ntj = 4 * gi + 4
                    pv = pv_ps.tile([128, 512], f32)
                    sm = sum_ps.tile([1, 512], f32)
                    for tj in range(ntj):
                        sc = score_ps.tile([128, 512], f32)
                        nc.tensor.matmul(
                            sc[:],
                            kT_sb[:, tj * 128:(tj + 1) * 128],
                            qh[:, gi * 512:gi * 512 + 512],
                            start=True,
                            stop=True,
                        )
                        pb = probs_pool.tile([128, 512], bf)
                        nc.scalar.activation(pb[:], sc[:], Exp, scale=inv_sqrt_d)
                        if tj >= 4 * gi:  # diagonal block: causal 0/1 mask
                            p = tj - 4 * gi
                            nc.vector.tensor_mul(
                                pb[:], pb[:], mask_sb[:, p * 512:(p + 1) * 512]
                            )
                        nc.tensor.matmul(
                            sm[:], ones_sb[:], pb[:],
                            start=(tj == 0), stop=(tj == ntj - 1),
                        )
                        nc.tensor.matmul(
                            pv[:], vn_sb[:, tj * 128:(tj + 1) * 128], pb[:],
                            start=(tj == 0), stop=(tj == ntj - 1),
                        )
                    rc = rc_pool.tile([1, 512], f32)
                    nc.vector.reciprocal(rc[:], sm[:])
                    # broadcast recip row to 128 partitions: ones[1,128].T @ rc[1,512]
                    bc_ps = mm_ps.tile([128, 512], f32, tag="mm")
                    nc.tensor.matmul(bc_ps[:], onerow_sb[:], rc[:], start=True, stop=True)
                    bc = bc_pool.tile([128, 512], f32)
                    nc.vector.tensor_copy(bc[:], bc_ps[:])
                    nc.vector.tensor_mul(
                        at_sb[:, h * S + gi * 512: h * S + gi * 512 + 512],
                        pv[:], bc[:],
                    )

            # ---- o_proj partial: out[s, e] = sum_h attnT_h[:, s].T @ woT_h ----
            for st in range(ST):
                for eg in range(SB):
                    ps = mm_ps.tile([128, 512], f32, tag="mm")
                    for h in range(HPC):
                        nc.tensor.matmul(
                            ps[:],
                            at_sb[:, h * S + st * 128: h * S + st * 128 + 128],
                            wo_sb[:, h * HID + eg * 512: h * HID + eg * 512 + 512],
                            start=(h == 0),
                            stop=(h == HPC - 1),
                        )
                    ostage = ostage_pool.tile([128, 512], f32)
                    nc.vector.tensor_copy(ostage[:], ps[:])
                    nc.sync.dma_start(
                        out[st * 128:(st + 1) * 128, eg * 512:(eg + 1) * 512],
                        ostage[:],
                    )
    _legalize_waits(nc)
    return nc


def _host_prep(hidden_states, Wq, Wk, Wv, Wo, position_ids):
    bf = ml_dtypes.bfloat16
    inv_freq = 1.0 / (THETA ** (np.arange(0, HD, 2, dtype=np.float64) / HD))

    mask = np.zeros((4, 128, 512), dtype=bf)
    jl = np.arange(128)[:, None]
    il = np.arange(512)[None, :]
    for p in range(4):
        mask[p] = (128 * p + jl <= il).astype(bf)

    in_maps = []
    for d in range(NCORES):
        b, g = d // 4, d % 4
        hsT = np.ascontiguousarray(hidden_states[b].T).astype(bf).reshape(KT, 128, S)
        wqT = np.ascontiguousarray(Wq[g * QDIM:(g + 1) * QDIM].T).astype(bf).reshape(KT, 128, QDIM)
        wkT = np.ascontiguousarray(Wk[g * HD:(g + 1) * HD].T).astype(bf).reshape(KT, 128, HD)
        wvT = np.ascontiguousarray(Wv[g * HD:(g + 1) * HD].T).astype(bf).reshape(KT, 128, HD)
        woT = np.ascontiguousarray(Wo[:, g * QDIM:(g + 1) * QDIM].T).astype(bf).reshape(4, 128, HID)
        freqs = position_ids[b].astype(np.float64)[:, None] * inv_freq[None, :]  # [S, 64]
        emb = np.concatenate([freqs, freqs], axis=1)  # [S, 128]
        cosT = np.cos(emb).T.astype(bf)
        sinT = np.sin(emb).T.astype(bf)
        in_maps.append({
            "hsT": hsT, "wqT": wqT, "wkT": wkT, "wvT": wvT, "woT": woT,
            "cosT": np.ascontiguousarray(cosT),
            "sinT": np.ascontiguousarray(sinT),
            "mask": mask,
        })
    return in_maps


def kernel(hidden_states, Wq, Wk, Wv, Wo, position_ids, _trace=False):
    from concourse.bass_utils import run_bass_kernel_spmd

    if "nc" not in _CACHE:
        _CACHE["nc"] = _build_nc()
    nc = _CACHE["nc"]

    in_maps = _host_prep(
        np.asarray(hidden_states), np.asarray(Wq), np.asarray(Wk),
        np.asarray(Wv), np.asarray(Wo), np.asarray(position_ids),
    )
    res = run_bass_kernel_spmd(nc, in_maps, core_ids=list(range(NCORES)), trace=_trace)
    _CACHE["last_result"] = res

    out = np.zeros((B, S, NH * HD), dtype=np.float32)
    for d in range(NCORES):
        out[d // 4] += res.results[d]["out"]
    return out


# revision 12
# speedup vs baseline: 13644.7625x; 13644.7625x over previous
"""Tensor-parallel Llama attention for 8 TRN2 NeuronCores.

Sharding: core d handles batch d//4 and q-head group g = d%4 (q heads
4g..4g+3, kv head g — GQA group-aligned so each core needs exactly one
kv head).  Wq/Wk/Wv are row-sharded, Wo column-sharded; the per-batch
partial o_proj outputs of 4 cores are summed on the host.

Device layouts (prepared host-side, bf16):
  hsT  [16,128,S]   hidden_states[b].T, HID on partitions in 16 chunks
  wqT  [16,128,512] Wq_shard.T          wkT/wvT [16,128,128]
  woT  [4,128,2048] Wo_shard.T (4 contraction chunks of the 512 local dims)
  cosT/sinT [128,S] RoPE tables in [head_dim, seq] layout
  mask [4,128,512]  0/1 causal masks for the 4 diagonal-block phases

Compute: q/k kept transposed [d, s] for scores; v transposed back to
[s, d] via PE transpose for PV; scores computed as scoresT [j, i] so
softmax probs feed PV directly without transposition.  Softmax sums via
ones-vector matmul over partitions (no max subtraction: inputs are
N(0,~0.8) scores, exp stays far below f32 overflow).
"""

import sys

sys.path.insert(0, "/opt/trn_rl_repo")

import numpy as np
import ml_dtypes

B, S, HID = 2, 2048, 2048
NH, NKV, HD = 16, 4, 128
THETA = 10000.0
NCORES = 8
HPC = 4            # q heads per core
QDIM = HPC * HD    # 512 local q dims
KT = HID // 128    # 16 contraction chunks
SB = S // 512      # 4 column groups of 512
ST = S // 128      # 16 row tiles of 128

_CACHE = {}


def _patch_tile_drain():
    """This walrus build caps sync waits per CTRL instruction below what the
    stock Tile kernel-tail drain carries; split them into single-wait NOPs."""
    import bass_rust
    import concourse.tile as tile
    from concourse.tile import ScopedClock

    if getattr(tile.TileContext, "_drain_split_patched", False):
        return

    def _split_drain_and_barrier(self, tick_clock, wait_clock):
        ticks = list(tick_clock.global_clock)
        for i, v in enumerate(ticks):
            if v > 0:
                single = [0] * len(ticks)
                single[i] = v
                nop = self.nc.sync.nop(nofuse=True, hint=f"drain_wait_{i}")
                wait_clock.add_sem_waits(
                    nop.ins, ScopedClock({None: bass_rust.VectorClock(single)})
                )
        self.nc.sync.drain()
        self.nc.all_engine_barrier()
        assert self.sems is not None
        popped = self.nc._tile_sem_poison_stack.pop()
        assert popped is self._sem_poison
        self.nc.clear_and_free_semaphores(list(self.sems.allocated().values()))
        self.nc.all_engine_barrier()

    tile.TileContext._drain_and_barrier = _split_drain_and_barrier
    tile.TileContext._drain_split_patched = True


def _legalize_waits(nc, max_waits=1):
    """This walrus build rejects instructions carrying more than ~2 sync
    waits.  Hoist the excess onto single-wait NOPs inserted just before the
    instruction in its block (same engine => same instruction stream, so
    the waits still complete before the op issues)."""
    import concourse.mybir as mybir

    n_split = 0
    for block in nc.m.functions[0].blocks:
        insts = list(block.instructions)
        out = []
        for inst in insts:
            si = getattr(inst, "sync_info", None)
            if si is not None and si.on_wait and len(si.on_wait) > max_waits:
                waits = list(si.on_wait)
                keep = waits[:max_waits]
                for j, w in enumerate(waits[max_waits:]):
                    out.append(
                        mybir.InstNoOp(
                            name=f"{inst.name}_hw{j}",
                            engine=inst.engine,
                            bass_nofuse=True,
                            sync_info=mybir.SyncInfo(on_wait=[w], on_update=[]),
                        )
                    )
                si.on_wait = keep
                n_split += 1
            out.append(inst)
        block.instructions = out
    return n_split


def _build_nc():
    import concourse.bass as bass
    import concourse.mybir as mybir
    import concourse.tile as tile
    from concourse.masks import make_identity

    _patch_tile_drain()

    bf = mybir.dt.bfloat16
    f32 = mybir.dt.float32
    Exp = mybir.ActivationFunctionType.Exp

    nc = bass.Bass()
    hsT = nc.declare_dram_parameter("hsT", [KT, 128, S], bf, isOutput=False)
    wqT = nc.declare_dram_parameter("wqT", [KT, 128, QDIM], bf, isOutput=False)
    wkT = nc.declare_dram_parameter("wkT", [KT, 128, HD], bf, isOutput=False)
    wvT = nc.declare_dram_parameter("wvT", [KT, 128, HD], bf, isOutput=False)
    woT = nc.declare_dram_parameter("woT", [4, 128, HID], bf, isOutput=False)
    cosT = nc.declare_dram_parameter("cosT", [128, S], bf, isOutput=False)
    sinT = nc.declare_dram_parameter("sinT", [128, S], bf, isOutput=False)
    mask = nc.declare_dram_parameter("mask", [4, 128, 512], bf, isOutput=False)
    out = nc.declare_dram_parameter("out", [S, HID], f32, isOutput=True)

    with tile.TileContext(nc) as tc:
        with (
            tc.tile_pool(name="resid", bufs=1) as resid,
            tc.tile_pool(name="probs", bufs=4) as probs_pool,
            tc.tile_pool(name="rc", bufs=2) as rc_pool,
            tc.tile_pool(name="bc", bufs=2) as bc_pool,
            tc.tile_pool(name="ostage", bufs=4) as ostage_pool,
            tc.tile_pool(name="mm_ps", bufs=2, space="PSUM") as mm_ps,
            tc.tile_pool(name="score_ps", bufs=2, space="PSUM") as score_ps,
            tc.tile_pool(name="pv_ps", bufs=2, space="PSUM") as pv_ps,
            tc.tile_pool(name="sum_ps", bufs=2, space="PSUM") as sum_ps,
        ):
            hs_sb = resid.tile([128, KT * S], bf)
            wq_sb = resid.tile([128, KT * QDIM], bf)
            wk_sb = resid.tile([128, KT * HD], bf)
            wv_sb = resid.tile([128, KT * HD], bf)
            wo_sb = resid.tile([128, 4 * HID], bf)
            cos_sb = resid.tile([128, S], bf)
            sin_sb = resid.tile([128, S], bf)
            mask_sb = resid.tile([128, 4 * 512], bf)
            ones_sb = resid.tile([128, 1], bf)
            onerow_sb = resid.tile([1, 128], f32)
            ident = resid.tile([128, 128], bf)
            qT_sb = resid.tile([128, HPC * S], bf)
            kT_sb = resid.tile([128, S], bf)
            vT_sb = resid.tile([128, S], bf)
            vn_sb = resid.tile([128, S], bf)
            at_sb = resid.tile([128, HPC * S], bf)
            rot_sb = resid.tile([128, S], bf)

            # ---- load everything ----
            for kk in range(KT):
                nc.sync.dma_start(hs_sb[:, kk * S:(kk + 1) * S], hsT[kk])
                nc.sync.dma_start(wq_sb[:, kk * QDIM:(kk + 1) * QDIM], wqT[kk])
                nc.sync.dma_start(wk_sb[:, kk * HD:(kk + 1) * HD], wkT[kk])
                nc.sync.dma_start(wv_sb[:, kk * HD:(kk + 1) * HD], wvT[kk])
            for c in range(4):
                nc.sync.dma_start(wo_sb[:, c * HID:(c + 1) * HID], woT[c])
                nc.sync.dma_start(mask_sb[:, c * 512:(c + 1) * 512], mask[c])
            nc.sync.dma_start(cos_sb[:], cosT[:])
            nc.sync.dma_start(sin_sb[:], sinT[:])
            nc.gpsimd.memset(ones_sb[:], 1.0)
            nc.gpsimd.memset(onerow_sb[:], 1.0)
            make_identity(nc, ident[:])

            # ---- q/k/v projections (transposed layouts) ----
            def project(w_sb, wdim, mtiles, dst, dst_stride):
                # dst[:, m*dst_stride + sg*512 ...] = (W.T chunk m) over s
                for m in range(mtiles):
                    for sg in range(SB):
                        ps = mm_ps.tile([128, 512], f32, tag="mm")
                        for kk in range(KT):
                            nc.tensor.matmul(
                                ps[:],
                                w_sb[:, kk * wdim + m * 128: kk * wdim + (m + 1) * 128],
                                hs_sb[:, kk * S + sg * 512: kk * S + sg * 512 + 512],
                                start=(kk == 0),
                                stop=(kk == KT - 1),
                            )
                        nc.vector.tensor_copy(
                            dst[:, m * dst_stride + sg * 512: m * dst_stride + sg * 512 + 512],
                            ps[:],
                        )

            project(wq_sb, QDIM, HPC, qT_sb, S)
            project(wk_sb, HD, 1, kT_sb, S)
            project(wv_sb, HD, 1, vT_sb, S)

            # ---- RoPE (in place, [d, s] layout) ----
            def rope(h):
                nc.vector.tensor_scalar_mul(rot_sb[0:64, :], h[64:128, :], -1.0)
                nc.vector.tensor_copy(rot_sb[64:128, :], h[0:64, :])
                nc.vector.tensor_mul(h, h, cos_sb[:])
                nc.vector.tensor_mul(rot_sb[:], rot_sb[:], sin_sb[:])
                nc.vector.tensor_add(h, h, rot_sb[:])

            for h in range(HPC):
                rope(qT_sb[:, h * S:(h + 1) * S])
            rope(kT_sb[:])

            # ---- v back to natural [s, d] layout via PE transpose ----
            for tj in range(ST):
                tp = mm_ps.tile([128, 128], bf, tag="mm")
                nc.tensor.transpose(tp[:], vT_sb[:, tj * 128:(tj + 1) * 128], ident[:])
                nc.vector.tensor_copy(vn_sb[:, tj * 128:(tj + 1) * 128], tp[:])

            # ---- attention: scoresT -> exp -> mask -> sums + PV ----
            inv_sqrt_d = 1.0 / float(np.sqrt(HD))
            for h in range(HPC):
                qh = qT_sb[:, h * S:(h + 1) * S]
                for gi in range(SB):
                    ntj = 4 * gi + 4
                    pv = pv_ps.tile([128, 512], f32)
                    sm = sum_ps.tile([1, 512], f32)
                    for tj in range(ntj):
                        sc = score_ps.tile([128, 512], f32)
                        nc.tensor.matmul(
                            sc[:],
                            kT_sb[:, tj * 128:(tj + 1) * 128],
                            qh[:, gi * 512:gi * 512 + 512],
                            start=True,
                            stop=True,
                        )
                        pb = probs_pool.tile([128, 512], bf)
                        nc.scalar.activation(pb[:], sc[:], Exp, scale=inv_sqrt_d)
                        if tj >= 4 * gi:  # diagonal block: causal 0/1 mask
                            p = tj - 4 * gi
                            nc.vector.tensor_mul(
                                pb[:], pb[:], mask_sb[:, p * 512:(p + 1) * 512]
                            )
                        nc.tensor.matmul(
                            sm[:], ones_sb[:], pb[:],
                            start=(tj == 0), stop=(tj == ntj - 1),
                        )
                        nc.tensor.matmul(
                            pv[:], vn_sb[:, tj * 128:(tj + 1) * 128], pb[:],
                            start=(tj == 0), stop=(tj == ntj - 1),
                        )
                    rc = rc_pool.tile([1, 512], f32)
                    nc.vector.reciprocal(rc[:], sm[:])
                    # broadcast recip row to 128 partitions: ones[1,128].T @ rc[1,512]
                    bc_ps = mm_ps.tile([128, 512], f32, tag="mm")
                    nc.tensor.matmul(bc_ps[:], onerow_sb[:], rc[:], start=True, stop=True)
                    bc = bc_pool.tile([128, 512], f32)
                    nc.vector.tensor_copy(bc[:], bc_ps[:])
                    nc.vector.tensor_mul(
                        at_sb[:, h * S + gi * 512: h * S + gi * 512 + 512],
                        pv[:], bc[:],
                    )

            # ---- o_proj partial: out[s, e] = sum_h attnT_h[:, s].T @ woT_h ----
            for st in range(ST):
                for eg in range(SB):
                    ps = mm_ps.tile([128, 512], f32, tag="mm")
                    for h in range(HPC):
                        nc.tensor.matmul(
                            ps[:],
                            at_sb[:, h * S + st * 128: h * S + st * 128 + 128],
                            wo_sb[:, h * HID + eg * 512: h * HID + eg * 512 + 512],
                            start=(h == 0),
                            stop=(h == HPC - 1),
                        )
                    ostage = ostage_pool.tile([128, 512], f32)
                    nc.vector.tensor_copy(ostage[:], ps[:])
                    nc.sync.dma_start(
                        out[st * 128:(st + 1) * 128, eg * 512:(eg + 1) * 512],
                        ostage[:],
                    )
    _legalize_waits(nc)
    return nc


def _host_prep(hidden_states, Wq, Wk, Wv, Wo, position_ids):
    bf = ml_dtypes.bfloat16
    inv_freq = 1.0 / (THETA ** (np.arange(0, HD, 2, dtype=np.float64) / HD))

    mask = np.zeros((4, 128, 512), dtype=bf)
    jl = np.arange(128)[:, None]
    il = np.arange(512)[None, :]
    for p in range(4):
        mask[p] = (128 * p + jl <= il).astype(bf)

    in_maps = []
    for d in range(NCORES):
        b, g = d // 4, d % 4
        hsT = np.ascontiguousarray(hidden_states[b].T).astype(bf).reshape(KT, 128, S)
        wqT = np.ascontiguousarray(Wq[g * QDIM:(g + 1) * QDIM].T).astype(bf).reshape(KT, 128, QDIM)
        wkT = np.ascontiguousarray(Wk[g * HD:(g + 1) * HD].T).astype(bf).reshape(KT, 128, HD)
        wvT = np.ascontiguousarray(Wv[g * HD:(g + 1) * HD].T).astype(bf).reshape(KT, 128, HD)
        woT = np.ascontiguousarray(Wo[:, g * QDIM:(g + 1) * QDIM].T).astype(bf).reshape(4, 128, HID)
        freqs = position_ids[b].astype(np.float64)[:, None] * inv_freq[None, :]  # [S, 64]
        emb = np.concatenate([freqs, freqs], axis=1)  # [S, 128]
        cosT = np.cos(emb).T.astype(bf)
        sinT = np.sin(emb).T.astype(bf)
        in_maps.append({
            "hsT": hsT, "wqT": wqT, "wkT": wkT, "wvT": wvT, "woT": woT,
            "cosT": np.ascontiguousarray(cosT),
            "sinT": np.ascontiguousarray(sinT),
            "mask": mask,
        })
    return in_maps


def kernel(hidden_states, Wq, Wk, Wv, Wo, position_ids, _trace=False, _tmpdir=None):
    from concourse.bass_utils import run_bass_kernel_spmd

    if "nc" not in _CACHE:
        _CACHE["nc"] = _build_nc()
    nc = _CACHE["nc"]

    in_maps = _host_prep(
        np.asarray(hidden_states), np.asarray(Wq), np.asarray(Wk),
        np.asarray(Wv), np.asarray(Wo), np.asarray(position_ids),
    )
    res = run_bass_kernel_spmd(
        nc, in_maps, core_ids=list(range(NCORES)), trace=_trace, tmpdir=_tmpdir
    )
    _CACHE["last_result"] = res

    out = np.zeros((B, S, NH * HD), dtype=np.float32)
    for d in range(NCORES):
        out[d // 4] += res.results[d]["out"]
    return out


# revision 19
# speedup vs baseline: 14655.7097x; 1.0741x over previous
"""Tensor-parallel Llama attention for 8 TRN2 NeuronCores.

Sharding: core d handles batch d//4 and q-head group g = d%4 (q heads
4g..4g+3, kv head g — GQA group-aligned so each core needs exactly one
kv head).  Wq/Wk/Wv are row-sharded, Wo column-sharded; the per-batch
partial o_proj outputs of 4 cores are summed on the host.

Device layouts (prepared host-side, bf16):
  hsT  [16,128,S]   hidden_states[b].T, HID on partitions in 16 chunks
  wqT  [16,128,512] Wq_shard.T          wkT/wvT [16,128,128]
  woT  [4,128,2048] Wo_shard.T (4 contraction chunks of the 512 local dims)
  cosT/sinT [128,S] RoPE tables in [head_dim, seq] layout
  mask [4,128,512]  0/1 causal masks for the 4 diagonal-block phases

Compute: q/k kept transposed [d, s] for scores; v transposed back to
[s, d] via PE transpose for PV; scores computed as scoresT [j, i] so
softmax probs feed PV directly without transposition.  Softmax sums via
ones-vector matmul over partitions (no max subtraction: inputs are
N(0,~0.8) scores, exp stays far below f32 overflow).
"""

import sys

sys.path.insert(0, "/opt/trn_rl_repo")

import numpy as np
import ml_dtypes

B, S, HID = 2, 2048, 2048
NH, NKV, HD = 16, 4, 128
THETA = 10000.0
NCORES = 8
HPC = 4            # q heads per core
QDIM = HPC * HD    # 512 local q dims
KT = HID // 128    # 16 contraction chunks
SB = S // 512      # 4 column groups of 512
ST = S // 128      # 16 row tiles of 128

_CACHE = {}


def _patch_tile_drain():
    """This walrus build caps sync waits per CTRL instruction below what the
    stock Tile kernel-tail drain carries; split them into single-wait NOPs."""
    import bass_rust
    import concourse.tile as tile
    from concourse.tile import ScopedClock

    if getattr(tile.TileContext, "_drain_split_patched", False):
        return

    def _split_drain_and_barrier(self, tick_clock, wait_clock):
        ticks = list(tick_clock.global_clock)
        for i, v in enumerate(ticks):
            if v > 0:
                single = [0] * len(ticks)
                single[i] = v
                nop = self.nc.sync.nop(nofuse=True, hint=f"drain_wait_{i}")
                wait_clock.add_sem_waits(
                    nop.ins, ScopedClock({None: bass_rust.VectorClock(single)})
                )
        self.nc.sync.drain()
        self.nc.all_engine_barrier()
        assert self.sems is not None
        popped = self.nc._tile_sem_poison_stack.pop()
        assert popped is self._sem_poison
        self.nc.clear_and_free_semaphores(list(self.sems.allocated().values()))
        self.nc.all_engine_barrier()

    tile.TileContext._drain_and_barrier = _split_drain_and_barrier
    tile.TileContext._drain_split_patched = True


def _legalize_waits(nc, max_waits=1):
    """This walrus build rejects instructions carrying more than ~2 sync
    waits.  Hoist the excess onto single-wait NOPs inserted just before the
    instruction in its block (same engine => same instruction stream, so
    the waits still complete before the op issues)."""
    import concourse.mybir as mybir

    n_split = 0
    for block in nc.m.functions[0].blocks:
        insts = list(block.instructions)
        out = []
        for inst in insts:
            si = getattr(inst, "sync_info", None)
            if si is not None and si.on_wait and len(si.on_wait) > max_waits:
                waits = list(si.on_wait)
                keep = waits[:max_waits]
                for j, w in enumerate(waits[max_waits:]):
                    out.append(
                        mybir.InstNoOp(
                            name=f"{inst.name}_hw{j}",
                            engine=inst.engine,
                            bass_nofuse=True,
                            sync_info=mybir.SyncInfo(on_wait=[w], on_update=[]),
                        )
                    )
                si.on_wait = keep
                n_split += 1
            out.append(inst)
        block.instructions = out
    return n_split


def _build_nc():
    import concourse.bass as bass
    import concourse.mybir as mybir
    import concourse.tile as tile
    from concourse.masks import make_identity

    _patch_tile_drain()

    bf = mybir.dt.bfloat16
    f32 = mybir.dt.float32
    Exp = mybir.ActivationFunctionType.Exp

    nc = bass.Bass()
    hsT = nc.declare_dram_parameter("hsT", [KT, 128, S], bf, isOutput=False)
    wqT = nc.declare_dram_parameter("wqT", [KT, 128, QDIM], bf, isOutput=False)
    wkT = nc.declare_dram_parameter("wkT", [KT, 128, HD], bf, isOutput=False)
    wvT = nc.declare_dram_parameter("wvT", [KT, 128, HD], bf, isOutput=False)
    woT = nc.declare_dram_parameter("woT", [4, 128, HID], bf, isOutput=False)
    cosT = nc.declare_dram_parameter("cosT", [128, S], bf, isOutput=False)
    sinT = nc.declare_dram_parameter("sinT", [128, S], bf, isOutput=False)
    mask = nc.declare_dram_parameter("mask", [4, 128, 512], bf, isOutput=False)
    out = nc.declare_dram_parameter("out", [S, HID], f32, isOutput=True)

    with tile.TileContext(nc) as tc:
        with (
            tc.tile_pool(name="resid", bufs=1) as resid,
            tc.tile_pool(name="probs", bufs=6) as probs_pool,
            tc.tile_pool(name="rc", bufs=2) as rc_pool,
            tc.tile_pool(name="bc", bufs=2) as bc_pool,
            tc.tile_pool(name="ostage", bufs=4) as ostage_pool,
            tc.tile_pool(name="mm_ps", bufs=2, space="PSUM") as mm_ps,
            tc.tile_pool(name="score_ps", bufs=2, space="PSUM") as score_ps,
            tc.tile_pool(name="pv_ps", bufs=2, space="PSUM") as pv_ps,
            tc.tile_pool(name="sum_ps", bufs=2, space="PSUM") as sum_ps,
        ):
            hs_sb = resid.tile([128, KT * S], bf)
            wq_sb = resid.tile([128, KT * QDIM], bf)
            wk_sb = resid.tile([128, KT * HD], bf)
            wv_sb = resid.tile([128, KT * HD], bf)
            wo_sb = resid.tile([128, 4 * HID], bf)
            cos_sb = resid.tile([128, S], bf)
            sin_sb = resid.tile([128, S], bf)
            mask_sb = resid.tile([128, 4 * 512], bf)
            ones_sb = resid.tile([128, 1], bf)
            ones4_sb = resid.tile([4, 128], f32)
            ident = resid.tile([128, 128], bf)
            qT_sb = resid.tile([128, HPC * S], bf)
            kT_sb = resid.tile([128, S], bf)
            vT_sb = resid.tile([128, S], bf)
            vn_sb = resid.tile([128, S], bf)
            at_sb = resid.tile([128, HPC * S], bf)
            rot_sb = resid.tile([128, S], bf)

            # ---- load everything ----
            for kk in range(KT):
                nc.sync.dma_start(hs_sb[:, kk * S:(kk + 1) * S], hsT[kk])
                nc.sync.dma_start(wq_sb[:, kk * QDIM:(kk + 1) * QDIM], wqT[kk])
                nc.sync.dma_start(wk_sb[:, kk * HD:(kk + 1) * HD], wkT[kk])
                nc.sync.dma_start(wv_sb[:, kk * HD:(kk + 1) * HD], wvT[kk])
            for c in range(4):
                nc.sync.dma_start(wo_sb[:, c * HID:(c + 1) * HID], woT[c])
                nc.sync.dma_start(mask_sb[:, c * 512:(c + 1) * 512], mask[c])
            nc.sync.dma_start(cos_sb[:], cosT[:])
            nc.sync.dma_start(sin_sb[:], sinT[:])
            nc.gpsimd.memset(ones_sb[:], 1.0)
            nc.gpsimd.memset(ones4_sb[:], 1.0)
            make_identity(nc, ident[:])

            # ---- q/k/v projections (transposed layouts) ----
            def project(w_sb, wdim, mtiles, dst, dst_stride):
                # dst[:, m*dst_stride + sg*512 ...] = (W.T chunk m) over s
                for m in range(mtiles):
                    for sg in range(SB):
                        ps = mm_ps.tile([128, 512], f32, tag="mm")
                        for kk in range(KT):
                            nc.tensor.matmul(
                                ps[:],
                                w_sb[:, kk * wdim + m * 128: kk * wdim + (m + 1) * 128],
                                hs_sb[:, kk * S + sg * 512: kk * S + sg * 512 + 512],
                                start=(kk == 0),
                                stop=(kk == KT - 1),
                            )
                        nc.vector.tensor_copy(
                            dst[:, m * dst_stride + sg * 512: m * dst_stride + sg * 512 + 512],
                            ps[:],
                        )

            # k/v first so attention can start while q heads still project
            project(wk_sb, HD, 1, kT_sb, S)
            project(wv_sb, HD, 1, vT_sb, S)

            # ---- v back to natural [s, d] layout via PE transpose ----
            for tj in range(ST):
                tp = mm_ps.tile([128, 128], bf, tag="mm")
                nc.tensor.transpose(tp[:], vT_sb[:, tj * 128:(tj + 1) * 128], ident[:])
                nc.vector.tensor_copy(vn_sb[:, tj * 128:(tj + 1) * 128], tp[:])

            project(wq_sb, QDIM, HPC, qT_sb, S)

            # ---- RoPE (in place, [d, s] layout); k first ----
            def rope(h):
                nc.vector.tensor_scalar_mul(rot_sb[0:64, :], h[64:128, :], -1.0)
                nc.vector.tensor_copy(rot_sb[64:128, :], h[0:64, :])
                nc.vector.tensor_mul(h, h, cos_sb[:])
                nc.vector.tensor_mul(rot_sb[:], rot_sb[:], sin_sb[:])
                nc.vector.tensor_add(h, h, rot_sb[:])

            rope(kT_sb[:])
            for h in range(HPC):
                rope(qT_sb[:, h * S:(h + 1) * S])

            # ---- attention (gi-outer so o_proj interleaves per i-group) ----
            inv_sqrt_d = 1.0 / float(np.sqrt(HD))
            for gi in range(SB):
                ntj = 4 * gi + 4
                pack = rc_pool.tile([128, 512], f32, tag="pack")
                for h in range(HPC):
                    qh = qT_sb[:, h * S:(h + 1) * S]
                    pv = pv_ps.tile([128, 512], f32)
                    sm = sum_ps.tile([1, 512], f32)
                    for tj in range(ntj):
                        sc = score_ps.tile([128, 512], f32)
                        nc.tensor.matmul(
                            sc[:],
                            kT_sb[:, tj * 128:(tj + 1) * 128],
                            qh[:, gi * 512:gi * 512 + 512],
                            start=True,
                            stop=True,
                        )
                        pb = probs_pool.tile([128, 512], bf)
                        nc.scalar.activation(pb[:], sc[:], Exp, scale=inv_sqrt_d)
                        if tj >= 4 * gi:  # diagonal block: causal 0/1 mask
                            p = tj - 4 * gi
                            nc.vector.tensor_mul(
                                pb[:], pb[:], mask_sb[:, p * 512:(p + 1) * 512]
                            )
                        nc.tensor.matmul(
                            sm[:], ones_sb[:], pb[:],
                            start=(tj == 0), stop=(tj == ntj - 1),
                        )
                        nc.tensor.matmul(
                            pv[:], vn_sb[:, tj * 128:(tj + 1) * 128], pb[:],
                            start=(tj == 0), stop=(tj == ntj - 1),
                        )
                    # stage unnormalized pv in at_sb; pack the sums row
                    nc.vector.tensor_copy(
                        at_sb[:, h * S + gi * 512: h * S + gi * 512 + 512], pv[:]
                    )
                    nc.vector.tensor_copy(pack[32 * h:32 * h + 1, :], sm[:])
                # one batched reciprocal for the 4 heads of this i-group
                # (rows live at 32-aligned partitions; other rows are junk)
                rcp = rc_pool.tile([128, 512], f32, tag="rcp")
                nc.vector.reciprocal(rcp[:], pack[:])
                for h in range(HPC):
                    rc1 = rc_pool.tile([1, 512], f32, tag="rc1")
                    nc.vector.tensor_copy(rc1[:], rcp[32 * h:32 * h + 1, :])
                    bc_ps = mm_ps.tile([128, 512], f32, tag="mm")
                    nc.tensor.matmul(
                        bc_ps[:], ones4_sb[0:1, :], rc1[:],
                        start=True, stop=True,
                    )
                    bc = bc_pool.tile([128, 512], f32)
                    nc.vector.tensor_copy(bc[:], bc_ps[:])
                    a_sl = at_sb[:, h * S + gi * 512: h * S + gi * 512 + 512]
                    nc.vector.tensor_mul(a_sl, a_sl, bc[:])
                # o_proj for the 4 s-tiles covered by this i-group
                for st in range(4 * gi, 4 * gi + 4):
                    for eg in range(SB):
                        ps = mm_ps.tile([128, 512], f32, tag="mm")
                        for h in range(HPC):
                            nc.tensor.matmul(
                                ps[:],
                                at_sb[:, h * S + st * 128: h * S + st * 128 + 128],
                                wo_sb[:, h * HID + eg * 512: h * HID + eg * 512 + 512],
                                start=(h == 0),
                                stop=(h == HPC - 1),
                            )
                        ostage = ostage_pool.tile([128, 512], f32)
                        nc.vector.tensor_copy(ostage[:], ps[:])
                        nc.sync.dma_start(
                            out[st * 128:(st + 1) * 128, eg * 512:(eg + 1) * 512],
                            ostage[:],
                        )
    _legalize_waits(nc)
    return nc


def _host_prep(hidden_states, Wq, Wk, Wv, Wo, position_ids):
    bf = ml_dtypes.bfloat16
    inv_freq = 1.0 / (THETA ** (np.arange(0, HD, 2, dtype=np.float64) / HD))

    mask = np.zeros((4, 128, 512), dtype=bf)
    jl = np.arange(128)[:, None]
    il = np.arange(512)[None, :]
    for p in range(4):
        mask[p] = (128 * p + jl <= il).astype(bf)

    in_maps = []
    for d in range(NCORES):
        b, g = d // 4, d % 4
        hsT = np.ascontiguousarray(hidden_states[b].T).astype(bf).reshape(KT, 128, S)
        wqT = np.ascontiguousarray(Wq[g * QDIM:(g + 1) * QDIM].T).astype(bf).reshape(KT, 128, QDIM)
        wkT = np.ascontiguousarray(Wk[g * HD:(g + 1) * HD].T).astype(bf).reshape(KT, 128, HD)
        wvT = np.ascontiguousarray(Wv[g * HD:(g + 1) * HD].T).astype(bf).reshape(KT, 128, HD)
        woT = np.ascontiguousarray(Wo[:, g * QDIM:(g + 1) * QDIM].T).astype(bf).reshape(4, 128, HID)
        freqs = position_ids[b].astype(np.float64)[:, None] * inv_freq[None, :]  # [S, 64]
        emb = np.concatenate([freqs, freqs], axis=1)  # [S, 128]
        cosT = np.cos(emb).T.astype(bf)
        sinT = np.sin(emb).T.astype(bf)
        in_maps.append({
            "hsT": hsT, "wqT": wqT, "wkT": wkT, "wvT": wvT, "woT": woT,
            "cosT": np.ascontiguousarray(cosT),
            "sinT": np.ascontiguousarray(sinT),
            "mask": mask,
        })
    return in_maps


def kernel(hidden_states, Wq, Wk, Wv, Wo, position_ids, _trace=False, _tmpdir=None):
    from concourse.bass_utils import run_bass_kernel_spmd

    if "nc" not in _CACHE:
        _CACHE["nc"] = _build_nc()
    nc = _CACHE["nc"]

    in_maps = _host_prep(
        np.asarray(hidden_states), np.asarray(Wq), np.asarray(Wk),
        np.asarray(Wv), np.asarray(Wo), np.asarray(position_ids),
    )
    res = run_bass_kernel_spmd(
        nc, in_maps, core_ids=list(range(NCORES)), trace=_trace, tmpdir=_tmpdir
    )
    _CACHE["last_result"] = res

    out = np.zeros((B, S, NH * HD), dtype=np.float32)
    for d in range(NCORES):
        out[d // 4] += res.results[d]["out"]
    return out
